# revision 3
# baseline (speedup 1.0000x reference)
"""Trainium2 Bass kernel for nn_Attn: out = softmax_s(v . (W @ q_s + b)).

Algebraic identity:
    energies[s] = v . (W @ q[s] + b) = q[s] . (W^T v) + (v . b)
The (v . b) term is constant and softmax is shift-invariant, so it drops out.
The kernel computes u = W^T v (tiny), energies = question @ u (a matvec), and
a sharded softmax.

Why NOT the PE array for the matvec: TensorE streams rhs at 1 fp32
column/SBUF-cycle and fp32 matmuls are 2-pass (LOW/HIGH), so pushing all of a
core's q through it costs ~2.3 ns per 128 elements (~75+ us/core) — well above
the ~47 us HBM floor. The DVE (vector engine) does a fused
multiply+free-dim-reduce (tensor_tensor_reduce) at 1 fp32 elem/lane/cycle
(0.96 GHz): a [128,1024] tile in ~1.1 us, 32 tiles in ~35 us — it hides under
the DMA stream.

Distribution over 8 NeuronCores — seq (token) sharding, question in its NATIVE
[tokens, H] layout (no host transpose):
  - core r owns tokens [r*4096, (r+1)*4096); partition p holds tokens
    [32p, 32p+32) of that slice, so every DMA is 128 partitions x contiguous
    bytes and the per-partition free axis is (t, h).
  - u = W^T v: core r computes u-slab r from W[:, 128r:128(r+1)] (512 KB,
    replicated v) on the otherwise-idle PE, AllGathers the 8 slabs (512 B
    each — mesh, ~5 us, fully overlapped with the q stream), and broadcasts
    u to all 128 partitions with a K=1 ones-matmul.
  - energies: 32 DVE tensor_tensor_reduce ops per core, one per 128-token
    tile: accum_out[:,t] = sum_h q_tile[:, t, h] * u_rep[:, h].
  - softmax: per-PARTITION stats only (no cross-partition reduction on
    device): negmax/rowsum via one DVE reduce + one ACT exp-with-accum, packed
    with the 32 unnormalized exp columns into one [128, 34] output DMA.
  - host merge (the standard sharded-softmax combine, O(S) data movement):
    global M = max m_rp, Sg = sum s_rp*exp(m_rp-M), out = p_un*exp(m_rp-M)/Sg.
"""

import numpy as np

S = 32768
H = 1024
NCORES = 8
TPC = S // NCORES  # 4096 tokens per core
TPT = 32  # tokens (sub-tiles) per partition
NCHUNK = 16  # 1 MB q DMAs per core
SPC = TPT // NCHUNK  # sub-tiles per chunk (2)
OC = H // 128  # 8 o-chunks for the u matmuls

_cached = {}


def _build():
    from contextlib import ExitStack

    import concourse.bass as bass
    import concourse.mybir as mybir
    import concourse.tile as tile
    from concourse import bacc

    f32 = mybir.dt.float32
    AX = mybir.AxisListType
    OP = mybir.AluOpType
    ds = bass.ds

    nc = bacc.Bacc(
        "TRN2", target_bir_lowering=False, debug=False, num_devices=NCORES
    )

    q = nc.dram_tensor("q", [TPC, H], f32, kind="ExternalInput")
    wc = nc.dram_tensor("wc", [H, 128], f32, kind="ExternalInput")
    vt = nc.dram_tensor("vt", [128, OC], f32, kind="ExternalInput")
    outp = nc.dram_tensor("outp", [128, TPT + 2], f32, kind="ExternalOutput")

    rg = [list(range(NCORES))]

    with tile.TileContext(nc) as tc, ExitStack() as ctx:
        const = ctx.enter_context(tc.tile_pool(name="const", bufs=1))
        qpool = ctx.enter_context(tc.tile_pool(name="qpool", bufs=NCHUNK))
        work = ctx.enter_context(tc.tile_pool(name="work", bufs=1))
        scr = ctx.enter_context(tc.tile_pool(name="scr", bufs=2))
        psum_u = ctx.enter_context(tc.tile_pool(name="psum_u", bufs=1, space="PSUM"))
        psum_b = ctx.enter_context(tc.tile_pool(name="psum_b", bufs=2, space="PSUM"))
        dram = ctx.enter_context(tc.tile_pool(name="dram", bufs=1, space="DRAM"))

        # tiny inputs on the ACT (scalar) HWDGE queue so the sync queue can
        # start streaming q at t=0
        v_sb = const.tile([128, OC], f32)
        nc.scalar.dma_start(v_sb[:], vt[:])
        wc_sb = const.tile([128, OC, 128], f32)
        nc.scalar.dma_start(wc_sb[:], wc[:].rearrange("(c p) m -> p c m", p=128))

        # ---- the q stream: 16 x 1 MB on the sync queue, back-to-back ----
        q_view = q[:].rearrange("(p t) h -> p (t h)", p=128)
        CW = SPC * H
        q_sb = []
        for k in range(NCHUNK):
            t_ = qpool.tile([128, CW], f32, tag="q")
            nc.sync.dma_start(t_[:], q_view[:, ds(k * CW, CW)])
            q_sb.append(t_)

        # ---- local u slab as a row: u_part[0, j] = sum_o W[o, 128r+j] v[o] ----
        pu = psum_u.tile([1, 128], f32, tag="pu")
        for c in range(OC):
            nc.tensor.matmul(
                pu[:], lhsT=v_sb[:, c : c + 1], rhs=wc_sb[:, c, :],
                start=(c == 0), stop=(c == OC - 1),
            )
        u_part = work.tile([1, 128], f32)
        nc.scalar.copy(u_part[:], pu[:])
        u_part_dram = dram.tile([1, 128], f32)
        nc.scalar.dma_start(u_part_dram[:], u_part[:])

        # ---- AllGather the 8 slabs; DRAM is linear so [1, H] holds them in
        # u-order (rank-major) ----
        u_all_dram = dram.tile([1, H], f32)
        nc.gpsimd.collective_compute(
            "AllGather", OP.bypass, replica_groups=rg,
            ins=[u_part_dram[:].opt()], outs=[u_all_dram.opt()],
        )
        u_row = work.tile([1, H], f32)
        nc.scalar.dma_start(u_row[:], u_all_dram[:])

        # ---- broadcast u across partitions: ones[1,128]^T @ u_row ----
        ones = const.tile([1, 128], f32)
        nc.gpsimd.memset(ones[:], 1.0)
        u_rep = const.tile([128, H], f32)
        for half in range(2):
            pb = psum_b.tile([128, 512], f32, tag="pb")
            nc.tensor.matmul(
                pb[:], lhsT=ones[:], rhs=u_row[:, ds(half * 512, 512)],
                start=True, stop=True,
            )
            nc.scalar.copy(u_rep[:, ds(half * 512, 512)], pb[:])

        # ---- energies: fused multiply + free-dim reduce on DVE ----
        # scalar_tensor_tensor: out = (in0 * 1.0) * in1, accum_out = sum(out)
        # (tensor_tensor_reduce crashes TRN2 hardware; this standard
        # InstTensorScalarPtr form is HW-verified)
        e_loc = work.tile([128, TPT], f32)
        for k in range(NCHUNK):
            for s_ in range(SPC):
                t_idx = k * SPC + s_
                prod = scr.tile([128, H], f32, tag="prod")
                nc.vector.scalar_tensor_tensor(
                    out=prod[:], in0=q_sb[k][:, ds(s_ * H, H)], scalar=1.0,
                    in1=u_rep[:], op0=OP.mult, op1=OP.mult,
                    accum_out=e_loc[:, ds(t_idx, 1)],
                )

        # ---- per-partition softmax pieces, packed with stats ----
        ot = work.tile([128, TPT + 2], f32)
        nc.vector.tensor_reduce(
            ot[:, ds(TPT, 1)], e_loc[:], axis=AX.X, op=OP.max, negate=True
        )
        nc.scalar.activation(
            ot[:, ds(0, TPT)], e_loc[:], mybir.ActivationFunctionType.Exp,
            bias=ot[:, ds(TPT, 1)], scale=1.0, accum_out=ot[:, ds(TPT + 1, 1)],
        )
        nc.sync.dma_start(outp[:], ot[:])

    nc.compile()
    return nc


def _get_nc():
    if "nc" not in _cached:
        _cached["nc"] = _build()
    return _cached["nc"]


def make_in_maps(question, W, v):
    qn = np.ascontiguousarray(np.asarray(question, dtype=np.float32))
    Wn = np.ascontiguousarray(np.asarray(W, dtype=np.float32))
    vn = np.ascontiguousarray(np.asarray(v, dtype=np.float32))
    vtf = np.ascontiguousarray(vn.reshape(OC, 128).T)  # vt[o, c] = v[128c + o]
    in_maps = []
    for r in range(NCORES):
        in_maps.append(
            {
                "q": qn[r * TPC : (r + 1) * TPC],  # contiguous row-slice view
                "wc": np.ascontiguousarray(Wn[:, r * 128 : (r + 1) * 128]),
                "vt": vtf,
            }
        )
    return in_maps


def run(question, W, v, **spmd_kwargs):
    """Run the SPMD kernel; returns (out [S] fp32, BassKernelResults)."""
    from concourse.bass_utils import run_bass_kernel_spmd

    nc = _get_nc()
    in_maps = make_in_maps(question, W, v)
    res = run_bass_kernel_spmd(nc, in_maps, core_ids=list(range(NCORES)), **spmd_kwargs)
    blocks = np.stack(
        [
            np.asarray(res.results[r]["outp"], dtype=np.float64).reshape(
                128, TPT + 2
            )
            for r in range(NCORES)
        ]
    )  # [8, 128, 34]; token of (r, p, t) = r*4096 + 32p + t
    p_un = blocks[:, :, :TPT]
    m = -blocks[:, :, TPT]
    sums = blocks[:, :, TPT + 1]
    M = m.max()
    wgt = np.exp(m - M)
    Sg = (sums * wgt).sum()
    out = (p_un * (wgt / Sg)[:, :, None]).reshape(S)
    return out.astype(np.float32), res


def kernel(question, W, b, v):
    out, _ = run(question, W, v)
    return out.reshape(1, 1, S)


# revision 5
# speedup vs baseline: 1.6593x; 1.6593x over previous
"""Trainium2 Bass kernel for nn_Attn: out = softmax_s(v . (W @ q_s + b)).

Algebraic identity:
    energies[s] = v . (W @ q[s] + b) = q[s] . (W^T v) + (v . b)
The (v . b) term is constant and softmax is shift-invariant, so it drops out.
The kernel computes u = W^T v (tiny), energies = question @ u (a matvec), and
a sharded softmax.

Why NOT the PE array for the matvec: TensorE streams rhs at 1 fp32
column/SBUF-cycle and fp32 matmuls are 2-pass (LOW/HIGH), so pushing all of a
core's q through it costs ~75+ us/core — well above the HBM floor. The DVE
does a fused multiply + free-axis-reduce (scalar_tensor_tensor with
accum_out) at 1 fp32 elem/lane/cycle: a [128,1024] tile in ~1.2 us, 32 tiles
in ~39 us — it hides under the DMA stream.

Why NO collectives: on this runner the 8 NEFFs enter ~60 us apart (the entry
barrier in every traced run spans to ~65 us), so ANY cross-core exchange
stalls the early cores for the skew. Instead every core reads the full W
(+4 MB on the stream, ~12 us) and computes u itself; cores are fully
independent, so each core's exec time is just its own work.

Distribution over 8 NeuronCores — seq (token) sharding, question in its
NATIVE [tokens, H] layout (no host transpose of the big tensor):
  - core r owns tokens [r*4096, (r+1)*4096); partition p holds tokens
    [32p, 32p+32), so every q DMA is 128 partitions x contiguous bytes.
  - q chunks are split across BOTH HWDGE queues (sync/SP and scalar/ACT) to
    get above the ~284 GB/s single-queue ceiling.
  - u: W is host-rearranged to [128 o-part, oc, j] so its DMA is
    128 x 32 KB contiguous; the otherwise-idle PE accumulates
    u = sum_oc W_oc^T v_oc into two [1,512] PSUM banks, pipelined with the
    4 W-piece DMAs; a K=1 ones-matmul broadcasts u to all 128 partitions.
  - energies: 32 DVE scalar_tensor_tensor ops, accum_out -> e_loc[:, t].
  - softmax: per-PARTITION stats only (negmax via DVE reduce, exp + rowsum
    via one ACT activation), packed with the 32 unnormalized exp columns
    into one [128, 34] output DMA.
  - host merge (standard sharded-softmax combine, O(S) data movement):
    M = max m_rp, Sg = sum s_rp*exp(m_rp-M), out = p_un * exp(m_rp-M)/Sg.
"""

import numpy as np

S = 32768
H = 1024
NCORES = 8
TPC = S // NCORES  # 4096 tokens per core
TPT = 32  # tokens (sub-tiles) per partition
NCHUNK = 16  # 1 MB q DMAs per core
SPC = TPT // NCHUNK  # sub-tiles per chunk (2)
OC = H // 128  # 8 o-chunks for the u matmuls
NWPC = 4  # W DMA pieces (1 MB each)
NQ_SYNC = 11  # q chunks on the sync queue; rest go on the scalar queue

_cached = {}


def _build():
    from contextlib import ExitStack

    import concourse.bass as bass
    import concourse.mybir as mybir
    import concourse.tile as tile
    from concourse import bacc

    f32 = mybir.dt.float32
    AX = mybir.AxisListType
    OP = mybir.AluOpType
    ds = bass.ds

    nc = bacc.Bacc(
        "TRN2", target_bir_lowering=False, debug=False, num_devices=NCORES
    )

    q = nc.dram_tensor("q", [TPC, H], f32, kind="ExternalInput")
    wcat = nc.dram_tensor("wcat", [128, OC * H], f32, kind="ExternalInput")
    vt = nc.dram_tensor("vt", [128, OC], f32, kind="ExternalInput")
    outp = nc.dram_tensor("outp", [128, TPT + 2], f32, kind="ExternalOutput")

    with tile.TileContext(nc) as tc, ExitStack() as ctx:
        const = ctx.enter_context(tc.tile_pool(name="const", bufs=1))
        qpool = ctx.enter_context(tc.tile_pool(name="qpool", bufs=NCHUNK))
        work = ctx.enter_context(tc.tile_pool(name="work", bufs=1))
        scr = ctx.enter_context(tc.tile_pool(name="scr", bufs=2))
        psum_u = ctx.enter_context(tc.tile_pool(name="psum_u", bufs=2, space="PSUM"))
        psum_b = ctx.enter_context(tc.tile_pool(name="psum_b", bufs=2, space="PSUM"))

        # --- scalar/ACT queue: v, W pieces first (u path), then tail q chunks
        v_sb = const.tile([128, OC], f32)
        nc.scalar.dma_start(v_sb[:], vt[:])
        w_sb = const.tile([128, OC * H], f32)
        WPW = OC * H // NWPC  # elems per W piece
        for w in range(NWPC):
            nc.scalar.dma_start(
                w_sb[:, ds(w * WPW, WPW)], wcat[:, ds(w * WPW, WPW)]
            )

        # --- q stream split across both HWDGE queues
        q_view = q[:].rearrange("(p t) h -> p (t h)", p=128)
        CW = SPC * H
        q_sb = []
        for k in range(NCHUNK):
            t_ = qpool.tile([128, CW], f32, tag="q")
            eng = nc.sync if k < NQ_SYNC else nc.scalar
            eng.dma_start(t_[:], q_view[:, ds(k * CW, CW)])
            q_sb.append(t_)

        # --- u = W^T v on the PE, pipelined with W arrival; two PSUM banks
        pu0 = psum_u.tile([1, 512], f32, tag="pu0")
        pu1 = psum_u.tile([1, 512], f32, tag="pu1")
        pu = [pu0, pu1]
        for c in range(OC):
            for half in range(2):
                nc.tensor.matmul(
                    pu[half][:],
                    lhsT=v_sb[:, c : c + 1],
                    rhs=w_sb[:, ds(c * H + half * 512, 512)],
                    start=(c == 0),
                    stop=(c == OC - 1),
                )
        u_row = work.tile([1, H], f32)
        for half in range(2):
            nc.scalar.copy(u_row[:, ds(half * 512, 512)], pu[half][:])

        # --- broadcast u across partitions: ones[1,128]^T @ u_row
        ones = const.tile([1, 128], f32)
        nc.gpsimd.memset(ones[:], 1.0)
        u_rep = const.tile([128, H], f32)
        for half in range(2):
            pb = psum_b.tile([128, 512], f32, tag="pb")
            nc.tensor.matmul(
                pb[:], lhsT=ones[:], rhs=u_row[:, ds(half * 512, 512)],
                start=True, stop=True,
            )
            nc.scalar.copy(u_rep[:, ds(half * 512, 512)], pb[:])

        # --- energies: fused multiply + free-axis reduce on DVE
        # out = (in0 * 1.0) * in1, accum_out = sum(out)
        e_loc = work.tile([128, TPT], f32)
        for k in range(NCHUNK):
            for s_ in range(SPC):
                t_idx = k * SPC + s_
                prod = scr.tile([128, H], f32, tag="prod")
                nc.vector.scalar_tensor_tensor(
                    out=prod[:], in0=q_sb[k][:, ds(s_ * H, H)], scalar=1.0,
                    in1=u_rep[:], op0=OP.mult, op1=OP.mult,
                    accum_out=e_loc[:, ds(t_idx, 1)],
                )

        # --- per-partition softmax pieces, packed with stats
        ot = work.tile([128, TPT + 2], f32)
        nc.vector.tensor_reduce(
            ot[:, ds(TPT, 1)], e_loc[:], axis=AX.X, op=OP.max, negate=True
        )
        nc.scalar.activation(
            ot[:, ds(0, TPT)], e_loc[:], mybir.ActivationFunctionType.Exp,
            bias=ot[:, ds(TPT, 1)], scale=1.0, accum_out=ot[:, ds(TPT + 1, 1)],
        )
        nc.sync.dma_start(outp[:], ot[:])

    nc.compile()
    return nc


def _get_nc():
    if "nc" not in _cached:
        _cached["nc"] = _build()
    return _cached["nc"]


def make_in_maps(question, W, v):
    qn = np.ascontiguousarray(np.asarray(question, dtype=np.float32))
    Wn = np.ascontiguousarray(np.asarray(W, dtype=np.float32))
    vn = np.ascontiguousarray(np.asarray(v, dtype=np.float32))
    # wcat[o, oc*H + j] = W[oc*128 + o, j] -> DMA is 128 x 32 KB contiguous
    wcat = np.ascontiguousarray(
        Wn.reshape(OC, 128, H).transpose(1, 0, 2).reshape(128, OC * H)
    )
    vtf = np.ascontiguousarray(vn.reshape(OC, 128).T)  # vt[o, c] = v[128c + o]
    in_maps = []
    for r in range(NCORES):
        in_maps.append(
            {
                "q": qn[r * TPC : (r + 1) * TPC],  # contiguous row-slice view
                "wcat": wcat,
                "vt": vtf,
            }
        )
    return in_maps


def run(question, W, v, **spmd_kwargs):
    """Run the SPMD kernel; returns (out [S] fp32, BassKernelResults)."""
    from concourse.bass_utils import run_bass_kernel_spmd

    nc = _get_nc()
    in_maps = make_in_maps(question, W, v)
    res = run_bass_kernel_spmd(nc, in_maps, core_ids=list(range(NCORES)), **spmd_kwargs)
    blocks = np.stack(
        [
            np.asarray(res.results[r]["outp"], dtype=np.float64).reshape(
                128, TPT + 2
            )
            for r in range(NCORES)
        ]
    )  # [8, 128, 34]; token of (r, p, t) = r*4096 + 32p + t
    p_un = blocks[:, :, :TPT]
    m = -blocks[:, :, TPT]
    sums = blocks[:, :, TPT + 1]
    M = m.max()
    wgt = np.exp(m - M)
    Sg = (sums * wgt).sum()
    out = (p_un * (wgt / Sg)[:, :, None]).reshape(S)
    return out.astype(np.float32), res


def kernel(question, W, b, v):
    out, _ = run(question, W, v)
    return out.reshape(1, 1, S)
